# revision 14
# baseline (speedup 1.0000x reference)
"""VRP attention-decoder greedy-decode kernel for Trainium2 (Bass/Tile).

kernel(**inputs) takes the FULL unsharded inputs (B=1024) and returns
(cost[B], ll[B]) matching reference.reference().

Design ("batch-on-partition"): 8 NeuronCores x 128 instances; instance ==
SBUF partition.  All GEMM precompute (K1 = ne@Wk1, V = ne@Wv,
K2' = ne@(Wk2 Wout^T), NW = ne@Wq_step[:E] + ge@Wq_fixed) runs on-device
on the TensorEngine in a prologue -- per node: PE transpose of
ne[:, n, :] to [E, inst], then 4 matmuls with that block stationary --
so the host ships only the raw inputs (~7MB/core instead of ~27MB/core
of precomputed tables; host does no GEMMs at all).  NW rows are written
to a device DRAM scratch table and gathered per decode step by
prev-node index alongside a small [xy|demand] row gather.  The per-step
attention einsums are per-instance batched matvecs -> elementwise
products + pairwise-tree reductions on DVE/GPSIMD, split across both
engines by free-dim ranges.  argmax runs on masked pre-tanh logits
(tanh monotone + positive scaling), softmax uses per-head max shift and
reciprocal normalization, tanh and sqrt are computed via exp/ln so a
single ACT table set is used in-loop.
"""

import numpy as np

B = 1024
NCORES = 8
BC = B // NCORES          # 128 instances per core == SBUF partitions
N_CUST = 100
N = N_CUST + 1            # 101
E = 128
H = 8
DH = 16
T = 2 * N                 # 202
CLIP = 10.0
ISD = 1.0 / np.sqrt(DH)
ISE = 1.0 / np.sqrt(E)
NEGBIG = -1.0e9
NE_CH = 4                 # nodes per ne streaming chunk

# dyn layout: [0:100] dem | [100:102] depot | [102:202] iota_nodes |
# [202] 101*i | [203:304] mask0 | [304] 1.0
DYNW = 305

_COMPILED = {}


def build_nc():
    import concourse.bass as bass
    import concourse.bacc as bacc
    import concourse.mybir as mybir
    from concourse.tile import TileContext
    from concourse import masks

    fp32 = mybir.dt.float32
    Alu = mybir.AluOpType
    Act = mybir.ActivationFunctionType

    nc = bacc.Bacc()

    ne_in = nc.dram_tensor("ne", [BC, N * E], fp32, kind="ExternalInput")
    ge_in = nc.dram_tensor("ge", [BC, E], fp32, kind="ExternalInput")
    wk1_in = nc.dram_tensor("wk1", [E, E], fp32, kind="ExternalInput")
    wv_in = nc.dram_tensor("wv", [E, E], fp32, kind="ExternalInput")
    wk2t_in = nc.dram_tensor("wk2t", [E, E], fp32, kind="ExternalInput")
    woutt_in = nc.dram_tensor("woutt", [E, E], fp32, kind="ExternalInput")
    wqf_in = nc.dram_tensor("wqf", [E, E], fp32, kind="ExternalInput")
    wqse_in = nc.dram_tensor("wqse", [E, E], fp32, kind="ExternalInput")
    wql_in = nc.dram_tensor("wql", [1, E], fp32, kind="ExternalInput")
    xyd_in = nc.dram_tensor("xyd", [BC * N, 4], fp32, kind="ExternalInput")
    dyn_in = nc.dram_tensor("dyn", [BC, DYNW], fp32, kind="ExternalInput")

    out_dram = nc.dram_tensor("out", [BC, 2], fp32, kind="ExternalOutput")

    with TileContext(nc) as tc:
        with (
            tc.tile_pool(name="tables", bufs=1) as tp,
            tc.tile_pool(name="state", bufs=1) as sp,
            tc.tile_pool(name="scratch", bufs=1) as cp,
            tc.tile_pool(name="nestream", bufs=3) as npool,
            tc.tile_pool(name="statpool", bufs=3) as stpool,
            tc.tile_pool(name="ppre", bufs=1, space="PSUM") as ppre,
            tc.tile_pool(name="ptp", bufs=2, space="PSUM") as ptp,
            tc.tile_pool(name="pmm", bufs=2, space="PSUM") as pmm,
            tc.tile_pool(name="dram", bufs=1, space="DRAM") as dpool,
        ):
            # ---- resident tables (155KB/partition), filled by prologue ----
            k1l = tp.tile([BC, H * N * DH], fp32)   # (h, n, d)
            vl = tp.tile([BC, H * DH * N], fp32)    # (h, d, n)
            k2l = tp.tile([BC, N * E], fp32)        # (n, e)
            nw_dram = dpool.tile([BC * N, E], fp32)

            # ---- small loads ----
            dyn = sp.tile([BC, DYNW], fp32)
            nc.sync.dma_start(out=dyn[:], in_=dyn_in[:])
            dem = dyn[:, 0:100]
            depot = dyn[:, 100:102]
            iota_nodes = dyn[:, 102:202]
            iota101 = dyn[:, 202:203]
            ones_col = dyn[:, 304:305]

            wk1 = tp.tile([E, E], fp32)
            nc.sync.dma_start(out=wk1[:], in_=wk1_in[:])
            wv = tp.tile([E, E], fp32)
            nc.sync.dma_start(out=wv[:], in_=wv_in[:])
            wk2t = tp.tile([E, E], fp32)
            nc.sync.dma_start(out=wk2t[:], in_=wk2t_in[:])
            woutt = tp.tile([E, E], fp32)
            nc.sync.dma_start(out=woutt[:], in_=woutt_in[:])
            wqf = tp.tile([E, E], fp32)
            nc.sync.dma_start(out=wqf[:], in_=wqf_in[:])
            wqse = tp.tile([E, E], fp32)
            nc.sync.dma_start(out=wqse[:], in_=wqse_in[:])
            wql = tp.tile([1, E], fp32)
            nc.sync.dma_start(out=wql[:], in_=wql_in[:])
            geb = tp.tile([BC, E], fp32)
            nc.sync.dma_start(out=geb[:], in_=ge_in[:])

            ident = tp.tile([E, E], fp32)
            masks.make_identity(nc, ident[:])
            ones1 = tp.tile([1, BC], fp32)
            nc.vector.memset(ones1[:], 1.0)

            # ---- prologue GEMMs ----
            # C = Wk2 @ Wout^T  (out[i,f] = sum_k Wk2[i,k] Wout[f,k])
            cps = ppre.tile([E, E], fp32, tag="cps")
            nc.tensor.matmul(out=cps[:], lhsT=wk2t[:], rhs=woutt[:], start=True, stop=True)
            crhs = tp.tile([E, E], fp32)
            nc.vector.tensor_copy(out=crhs[:], in_=cps[:])

            # wrep = ones(BC) x Wq_step[E]  (broadcast last row across partitions)
            wrp = ppre.tile([BC, E], fp32, tag="wrp")
            nc.tensor.matmul(out=wrp[:], lhsT=ones1[:], rhs=wql[:], start=True, stop=True)
            wrep = tp.tile([BC, E], fp32)
            nc.vector.tensor_copy(out=wrep[:], in_=wrp[:])

            # Qf = ge @ Wq_fixed
            gept = ppre.tile([E, BC], fp32, tag="gept")
            nc.tensor.transpose(out=gept[:], in_=geb[:], identity=ident[:])
            gets = tp.tile([E, BC], fp32)
            nc.scalar.activation(out=gets[:], in_=gept[:], func=Act.Copy)
            qfp = ppre.tile([BC, E], fp32, tag="qfp")
            nc.tensor.matmul(out=qfp[:], lhsT=gets[:], rhs=wqf[:], start=True, stop=True)
            qf = tp.tile([BC, E], fp32)
            nc.vector.tensor_copy(out=qf[:], in_=qfp[:])

            # per-node: transpose ne_n -> stationary; 4 matmuls; scatter to
            # table layouts; NW row (+Qf) out to DRAM gather table.
            k1v4 = k1l[:].rearrange("p (h n d) -> p h n d", h=H, n=N)
            vlv4 = vl[:].rearrange("p (h d n) -> p h d n", h=H, d=DH)
            nwv = nw_dram[:].rearrange("(b n) e -> b n e", n=N)
            for c0 in range(0, N, NE_CH):
                w = min(NE_CH, N - c0)
                neb = npool.tile([BC, NE_CH * E], fp32, tag="neb")
                nc.sync.dma_start(out=neb[:, 0:w * E], in_=ne_in[:, c0 * E:(c0 + w) * E])
                for k in range(w):
                    n = c0 + k
                    tps = ptp.tile([E, BC], fp32, tag="tps")
                    nc.tensor.transpose(out=tps[:], in_=neb[:, k * E:(k + 1) * E], identity=ident[:])
                    stat = stpool.tile([E, BC], fp32, tag="stat")
                    nc.scalar.activation(out=stat[:], in_=tps[:], func=Act.Copy)
                    mo = pmm.tile([BC, 4 * E], fp32, tag="mo")
                    nc.tensor.matmul(out=mo[:, 0:E], lhsT=stat[:], rhs=wk1[:], start=True, stop=True)
                    nc.tensor.matmul(out=mo[:, E:2 * E], lhsT=stat[:], rhs=wv[:], start=True, stop=True)
                    nc.tensor.matmul(out=mo[:, 2 * E:3 * E], lhsT=stat[:], rhs=crhs[:], start=True, stop=True)
                    nc.tensor.matmul(out=mo[:, 3 * E:4 * E], lhsT=stat[:], rhs=wqse[:], start=True, stop=True)
                    nc.vector.tensor_copy(
                        out=k1v4[:, :, n, :],
                        in_=mo[:, 0:E].rearrange("p (h d) -> p h d", h=H))
                    nc.scalar.activation(
                        out=vlv4[:, :, :, n],
                        in_=mo[:, E:2 * E].rearrange("p (h d) -> p h d", h=H),
                        func=Act.Copy)
                    nc.vector.tensor_copy(out=k2l[:, n * E:(n + 1) * E], in_=mo[:, 2 * E:3 * E])
                    nwst = stpool.tile([BC, E], fp32, tag="nwst")
                    nc.vector.tensor_tensor(out=nwst[:], in0=mo[:, 3 * E:4 * E], in1=qf[:], op=Alu.add)
                    nc.scalar.dma_start(out=nwv[:, n, :], in_=nwst[:])

            # ---- state ----
            maskneg = sp.tile([BC, N], fp32)
            nc.vector.tensor_copy(out=maskneg[:], in_=dyn[:, 203:304])
            visited = sp.tile([BC, N_CUST], fp32)
            nc.vector.memset(visited[:], 0.0)
            Dcap = sp.tile([BC, 1], fp32)
            nc.vector.tensor_copy(out=Dcap[:], in_=ones_col)
            llacc = sp.tile([BC, 1], fp32)
            nc.vector.memset(llacc[:], 0.0)
            costacc = sp.tile([BC, 1], fp32)
            prevxy = sp.tile([BC, 2], fp32)
            nc.vector.tensor_copy(out=prevxy[:], in_=depot)
            idx_f = sp.tile([BC, 1], fp32)
            nc.vector.tensor_copy(out=idx_f[:], in_=iota101)
            idx_u = sp.tile([BC, 1], mybir.dt.uint32)
            nc.vector.tensor_copy(out=idx_u[:], in_=idx_f[:])
            prev_f = sp.tile([BC, 1], fp32)
            nc.vector.memset(prev_f[:], 0.0)
            idx_g = sp.tile([BC, 1], mybir.dt.uint32)
            nc.gpsimd.tensor_copy(out=idx_g[:], in_=idx_u[:])

            # ---- shared per-step scratch (~38KB/partition) ----
            g128 = cp.tile([BC, E], fp32, tag="g128")
            g4 = cp.tile([BC, 4], fp32, tag="g4")
            q1 = cp.tile([BC, E], fp32, tag="q1")
            dterm = cp.tile([BC, E], fp32, tag="dterm")
            prod = cp.tile([BC, 3328], fp32, tag="prod")
            ta = cp.tile([BC, 1664], fp32, tag="ta")
            tb = cp.tile([BC, 832], fp32, tag="tb")
            tc_ = cp.tile([BC, 416], fp32, tag="tc_")
            td = cp.tile([BC, 232], fp32, tag="td")
            te = cp.tile([BC, 128], fp32, tag="te")
            tf = cp.tile([BC, 64], fp32, tag="tf")
            scor = cp.tile([BC, H * N], fp32, tag="scor")
            uexp = cp.tile([BC, H * N], fp32, tag="uexp")
            ssum = cp.tile([BC, H], fp32, tag="ssum")
            srec = cp.tile([BC, H], fp32, tag="srec")
            nsc = cp.tile([BC, H], fp32, tag="nsc")
            hmax = cp.tile([BC, H], fp32, tag="hmax")
            glm = cp.tile([BC, E], fp32, tag="glm")
            raw = cp.tile([BC, N], fp32, tag="raw")
            mx8 = cp.tile([BC, 8], fp32, tag="mx8")
            nxt8 = cp.tile([BC, 8], mybir.dt.uint32, tag="nxt8")
            nxt_f = cp.tile([BC, 1], fp32, tag="nxt_f")
            ltan = cp.tile([BC, N], fp32, tag="ltan")
            lexp = cp.tile([BC, N], fp32, tag="lexp")
            lsum = cp.tile([BC, 1], fp32, tag="lsum")
            lmax = cp.tile([BC, 1], fp32, tag="lmax")
            nlmax = cp.tile([BC, 1], fp32, tag="nlmax")
            tiny = cp.tile([BC, 2], fp32, tag="tiny")
            seg = cp.tile([BC, 1], fp32, tag="seg")
            oh = cp.tile([BC, N_CUST], fp32, tag="oh")
            gtd = cp.tile([BC, N_CUST], fp32, tag="gtd")
            sdep = cp.tile([BC, 1], fp32, tag="sdep")
            sdep_i = cp.tile([BC, 1], mybir.dt.int32, tag="sdep_i")
            av = cp.tile([BC, 1], fp32, tag="av")
            dnew = cp.tile([BC, 1], fp32, tag="dnew")

            def dist_to(xyap, acc):
                nc.vector.tensor_tensor(out=tiny[:], in0=xyap, in1=prevxy[:], op=Alu.subtract)
                nc.vector.tensor_tensor(out=tiny[:], in0=tiny[:], in1=tiny[:], op=Alu.mult)
                nc.vector.tensor_reduce(out=seg[:], in_=tiny[:, None, :], axis=mybir.AxisListType.X, op=Alu.add)
                nc.vector.tensor_scalar(out=seg[:], in0=seg[:], scalar1=1e-10, scalar2=None, op0=Alu.add)
                nc.scalar.activation(out=seg[:], in_=seg[:], func=Act.Ln)
                nc.scalar.activation(out=seg[:], in_=seg[:], func=Act.Exp, bias=0.0, scale=0.5)
                nc.vector.tensor_tensor(out=acc[:], in0=acc[:], in1=seg[:], op=Alu.add)

            def step_body():
                # 1) gather [xy | dem] and Q1-part rows by prev (last-selected) index
                nc.gpsimd.indirect_dma_start(
                    out=g4[:], out_offset=None, in_=xyd_in[:],
                    in_offset=bass.IndirectOffsetOnAxis(ap=idx_g[:, :1], axis=0))
                nc.gpsimd.indirect_dma_start(
                    out=g128[:], out_offset=None, in_=nw_dram[:],
                    in_offset=bass.IndirectOffsetOnAxis(ap=idx_g[:, :1], axis=0))

                # 1b) deferred env update for the node selected last step.
                #     At t=0 prev=depot and this exactly reproduces the
                #     reference initial state (given visited=0, D=1).
                nc.vector.tensor_scalar(out=sdep[:], in0=prev_f[:], scalar1=0.0, scalar2=None, op0=Alu.is_equal)
                nc.vector.tensor_copy(out=sdep_i[:], in_=sdep[:])
                nc.vector.tensor_tensor(out=dnew[:], in0=Dcap[:], in1=g4[:, 2:3], op=Alu.subtract)
                nc.vector.select(out=Dcap[:], mask=sdep_i[:], on_true=ones_col, on_false=dnew[:])
                nc.vector.tensor_scalar(out=oh[:], in0=iota_nodes, scalar1=prev_f[:, :1], scalar2=None, op0=Alu.is_equal)
                nc.vector.tensor_tensor(out=visited[:], in0=visited[:], in1=oh[:], op=Alu.max)
                nc.vector.tensor_scalar(out=gtd[:], in0=dem, scalar1=Dcap[:, :1], scalar2=None, op0=Alu.is_gt)
                nc.vector.tensor_tensor(out=gtd[:], in0=gtd[:], in1=visited[:], op=Alu.max)
                nc.vector.tensor_scalar(out=maskneg[:, 1:N], in0=gtd[:], scalar1=float(NEGBIG), scalar2=None, op0=Alu.mult)
                nc.vector.tensor_reduce(out=av[:], in_=visited[:], axis=mybir.AxisListType.X, op=Alu.min)
                nc.vector.tensor_scalar(out=av[:], in0=av[:], scalar1=-1.0, scalar2=1.0, op0=Alu.mult, op1=Alu.add)
                nc.vector.tensor_tensor(out=av[:], in0=av[:], in1=sdep[:], op=Alu.mult)
                nc.vector.tensor_scalar(out=maskneg[:, 0:1], in0=av[:], scalar1=float(NEGBIG), scalar2=None, op0=Alu.mult)

                # 1c) deferred cost segment to the last-selected node
                dist_to(g4[:, 0:2], costacc)
                nc.vector.tensor_copy(out=prevxy[:], in_=g4[:, 0:2])

                # 2) Q1 = gathered + D * w_last
                nc.vector.tensor_scalar(out=dterm[:], in0=wrep[:], scalar1=Dcap[:, :1],
                                        scalar2=None, op0=Alu.mult)
                nc.vector.tensor_tensor(out=q1[:], in0=g128[:], in1=dterm[:], op=Alu.add)

                # 3) scores, head-pair chunks: K1L[h,n,d]*Q1[h,d] -> sum_d
                q1v = q1[:].rearrange("p (h d) -> p h d", h=H)
                k1v = k1l[:].rearrange("p (h n d) -> p h n d", h=H, n=N)
                p1v = prod[:, 0:2 * N * DH].rearrange("p (h n d) -> p h n d", h=2, n=N)
                for hp in range(4):
                    h0 = 2 * hp
                    qs = q1v[:, h0:h0 + 2, None, :].to_broadcast([BC, 2, 68, DH])
                    nc.vector.tensor_tensor(out=p1v[:, :, 0:68, :],
                                            in0=k1v[:, h0:h0 + 2, 0:68, :], in1=qs, op=Alu.mult)
                    qs2 = q1v[:, h0:h0 + 2, None, :].to_broadcast([BC, 2, 33, DH])
                    nc.gpsimd.tensor_tensor(out=p1v[:, :, 68:N, :],
                                            in0=k1v[:, h0:h0 + 2, 68:N, :], in1=qs2, op=Alu.mult)
                    a = prod[:, 0:2 * N * DH].rearrange("p (x d) -> p x d", d=DH)   # x=202
                    r1 = ta[:, 0:202 * 8].rearrange("p (x d) -> p x d", d=8)
                    nc.vector.tensor_tensor(out=r1[:, 0:140, :], in0=a[:, 0:140, 0:8], in1=a[:, 0:140, 8:16], op=Alu.add)
                    nc.gpsimd.tensor_tensor(out=r1[:, 140:202, :], in0=a[:, 140:202, 0:8], in1=a[:, 140:202, 8:16], op=Alu.add)
                    r2 = tb[:, 0:202 * 4].rearrange("p (x d) -> p x d", d=4)
                    nc.vector.tensor_tensor(out=r2[:, 0:140, :], in0=r1[:, 0:140, 0:4], in1=r1[:, 0:140, 4:8], op=Alu.add)
                    nc.gpsimd.tensor_tensor(out=r2[:, 140:202, :], in0=r1[:, 140:202, 0:4], in1=r1[:, 140:202, 4:8], op=Alu.add)
                    r3 = tc_[:, 0:202 * 2].rearrange("p (x d) -> p x d", d=2)
                    nc.vector.tensor_tensor(out=r3[:, :, :], in0=r2[:, :, 0:2], in1=r2[:, :, 2:4], op=Alu.add)
                    nc.vector.tensor_tensor(
                        out=scor[:, h0 * N:(h0 + 2) * N].rearrange("p (x o) -> p x o", o=1),
                        in0=r3[:, :, 0:1], in1=r3[:, :, 1:2], op=Alu.add)

                # 4) mask + per-head exp (accumulating denominator) + reciprocal
                nc.vector.tensor_tensor(
                    out=scor[:].rearrange("p (h n) -> p h n", h=H),
                    in0=scor[:].rearrange("p (h n) -> p h n", h=H),
                    in1=maskneg[:, None, :].to_broadcast([BC, H, N]), op=Alu.add)
                nc.vector.tensor_reduce(
                    out=hmax[:], in_=scor[:].rearrange("p (h n) -> p h n", h=H),
                    axis=mybir.AxisListType.X, op=Alu.max)
                nc.vector.tensor_scalar(out=hmax[:], in0=hmax[:], scalar1=float(-ISD), scalar2=None, op0=Alu.mult)
                for h in range(H):
                    nc.scalar.activation(out=uexp[:, h * N:(h + 1) * N],
                                         in_=scor[:, h * N:(h + 1) * N],
                                         func=Act.Exp, bias=hmax[:, h:h + 1], scale=float(ISD),
                                         accum_out=ssum[:, h:h + 1])
                nc.vector.reciprocal(out=srec[:], in_=ssum[:])
                nc.vector.tensor_tensor(out=nsc[:], in0=ssum[:], in1=srec[:], op=Alu.mult)
                nc.vector.tensor_scalar(out=nsc[:], in0=nsc[:], scalar1=-1.0, scalar2=2.0, op0=Alu.mult, op1=Alu.add)
                nc.vector.tensor_tensor(out=srec[:], in0=srec[:], in1=nsc[:], op=Alu.mult)

                # 5) glimpse, head-pair chunks: VL[h,d,n]*U[h,n] -> sum_n
                vlv = vl[:].rearrange("p (h d n) -> p h d n", h=H, d=DH)
                uv = uexp[:].rearrange("p (h n) -> p h n", h=H)
                p2v = prod[:, 0:2 * DH * N].rearrange("p (h d n) -> p h d n", h=2, d=DH)
                for hp in range(4):
                    h0 = 2 * hp
                    us = uv[:, h0:h0 + 2, None, 0:68].to_broadcast([BC, 2, DH, 68])
                    nc.vector.tensor_tensor(out=p2v[:, :, :, 0:68],
                                            in0=vlv[:, h0:h0 + 2, :, 0:68], in1=us, op=Alu.mult)
                    us2 = uv[:, h0:h0 + 2, None, 68:N].to_broadcast([BC, 2, DH, 33])
                    nc.gpsimd.tensor_tensor(out=p2v[:, :, :, 68:N],
                                            in0=vlv[:, h0:h0 + 2, :, 68:N], in1=us2, op=Alu.mult)
                    # n-tree: 101 -> 51 -> 26 -> 13 -> 7 -> 4 -> 2 -> 1  (x = 32 rows)
                    a = prod[:, 0:2 * DH * N].rearrange("p (x n) -> p x n", n=N)
                    r1 = ta[:, 0:32 * 51].rearrange("p (x n) -> p x n", n=51)
                    nc.vector.tensor_tensor(out=r1[:, 0:20, 0:50], in0=a[:, 0:20, 0:50], in1=a[:, 0:20, 50:100], op=Alu.add)
                    nc.gpsimd.tensor_tensor(out=r1[:, 20:32, 0:50], in0=a[:, 20:32, 0:50], in1=a[:, 20:32, 50:100], op=Alu.add)
                    nc.vector.tensor_copy(out=r1[:, :, 50:51], in_=a[:, :, 100:101])
                    r2 = tb[:, 0:32 * 26].rearrange("p (x n) -> p x n", n=26)
                    nc.vector.tensor_tensor(out=r2[:, :, 0:25], in0=r1[:, :, 0:25], in1=r1[:, :, 25:50], op=Alu.add)
                    nc.vector.tensor_copy(out=r2[:, :, 25:26], in_=r1[:, :, 50:51])
                    r3 = tc_[:, 0:32 * 13].rearrange("p (x n) -> p x n", n=13)
                    nc.vector.tensor_tensor(out=r3[:, :, :], in0=r2[:, :, 0:13], in1=r2[:, :, 13:26], op=Alu.add)
                    r4 = td[:, 0:32 * 7].rearrange("p (x n) -> p x n", n=7)
                    nc.vector.tensor_tensor(out=r4[:, :, 0:6], in0=r3[:, :, 0:6], in1=r3[:, :, 6:12], op=Alu.add)
                    nc.vector.tensor_copy(out=r4[:, :, 6:7], in_=r3[:, :, 12:13])
                    r5 = te[:, 0:32 * 4].rearrange("p (x n) -> p x n", n=4)
                    nc.vector.tensor_tensor(out=r5[:, :, 0:3], in0=r4[:, :, 0:3], in1=r4[:, :, 3:6], op=Alu.add)
                    nc.vector.tensor_copy(out=r5[:, :, 3:4], in_=r4[:, :, 6:7])
                    r6 = tf[:, 0:32 * 2].rearrange("p (x n) -> p x n", n=2)
                    nc.vector.tensor_tensor(out=r6[:, :, :], in0=r5[:, :, 0:2], in1=r5[:, :, 2:4], op=Alu.add)
                    nc.vector.tensor_tensor(
                        out=glm[:, h0 * DH:(h0 + 2) * DH].rearrange("p (x o) -> p x o", o=1),
                        in0=r6[:, :, 0:1], in1=r6[:, :, 1:2], op=Alu.add)
                # normalize glimpse per head
                nc.vector.tensor_tensor(
                    out=glm[:].rearrange("p (h d) -> p h d", h=H),
                    in0=glm[:].rearrange("p (h d) -> p h d", h=H),
                    in1=srec[:, :, None].to_broadcast([BC, H, DH]), op=Alu.mult)

                # 6) logits, n'-chunks of 26: K2L[n',e]*G[e] -> sum_e
                k2v = k2l[:].rearrange("p (n e) -> p n e", n=N)
                for c in range(4):
                    n0 = 26 * c
                    n1 = min(N, n0 + 26)
                    w = n1 - n0
                    gb = glm[:, None, :].to_broadcast([BC, w, E])
                    p3v = prod[:, 0:w * E].rearrange("p (n e) -> p n e", e=E)
                    nc.vector.tensor_tensor(out=p3v[:, :, :], in0=k2v[:, n0:n1, :], in1=gb, op=Alu.mult)
                    r1 = ta[:, 0:w * 64].rearrange("p (n e) -> p n e", e=64)
                    hw = (w * 2) // 3
                    nc.vector.tensor_tensor(out=r1[:, 0:hw, :], in0=p3v[:, 0:hw, 0:64], in1=p3v[:, 0:hw, 64:128], op=Alu.add)
                    nc.gpsimd.tensor_tensor(out=r1[:, hw:w, :], in0=p3v[:, hw:w, 0:64], in1=p3v[:, hw:w, 64:128], op=Alu.add)
                    r2 = tb[:, 0:w * 32].rearrange("p (n e) -> p n e", e=32)
                    nc.vector.tensor_tensor(out=r2[:, :, :], in0=r1[:, :, 0:32], in1=r1[:, :, 32:64], op=Alu.add)
                    r3 = tc_[:, 0:w * 16].rearrange("p (n e) -> p n e", e=16)
                    nc.vector.tensor_tensor(out=r3[:, :, :], in0=r2[:, :, 0:16], in1=r2[:, :, 16:32], op=Alu.add)
                    r4 = td[:, 0:w * 8].rearrange("p (n e) -> p n e", e=8)
                    nc.vector.tensor_tensor(out=r4[:, :, :], in0=r3[:, :, 0:8], in1=r3[:, :, 8:16], op=Alu.add)
                    r5 = te[:, 0:w * 4].rearrange("p (n e) -> p n e", e=4)
                    nc.vector.tensor_tensor(out=r5[:, :, :], in0=r4[:, :, 0:4], in1=r4[:, :, 4:8], op=Alu.add)
                    r6 = tf[:, 0:w * 2].rearrange("p (n e) -> p n e", e=2)
                    nc.vector.tensor_tensor(out=r6[:, :, :], in0=r5[:, :, 0:2], in1=r5[:, :, 2:4], op=Alu.add)
                    nc.vector.tensor_tensor(
                        out=raw[:, n0:n1].rearrange("p (n o) -> p n o", o=1),
                        in0=r6[:, :, 0:1], in1=r6[:, :, 1:2], op=Alu.add)

                # 7) mask + argmax on pre-tanh logits
                nc.vector.tensor_tensor(out=raw[:], in0=raw[:], in1=maskneg[:], op=Alu.add)
                nc.vector.max(out=mx8[:], in_=raw[:])
                nc.vector.max_index(out=nxt8[:], in_max=mx8[:], in_values=raw[:])
                nc.vector.tensor_copy(out=nxt_f[:], in_=nxt8[:, 0:1])

                # 8) ll: L = CLIP*tanh(ISE*rawu) + maskNEG; tanh via exp.
                nc.vector.tensor_tensor(out=ltan[:], in0=raw[:], in1=maskneg[:], op=Alu.subtract)
                nc.scalar.activation(out=lexp[:], in_=ltan[:], func=Act.Exp,
                                     bias=0.0, scale=float(2.0 * ISE))
                nc.vector.tensor_scalar(out=lexp[:], in0=lexp[:], scalar1=1.0, scalar2=None, op0=Alu.add)
                nc.vector.reciprocal(out=lexp[:], in_=lexp[:])
                nc.vector.tensor_scalar(out=ltan[:], in0=lexp[:], scalar1=-2.0 * CLIP, scalar2=CLIP, op0=Alu.mult, op1=Alu.add)
                nc.vector.tensor_tensor(out=ltan[:], in0=ltan[:], in1=maskneg[:], op=Alu.add)
                nc.vector.tensor_reduce(out=lmax[:], in_=ltan[:], axis=mybir.AxisListType.X, op=Alu.max)
                nc.vector.tensor_scalar(out=nlmax[:], in0=lmax[:], scalar1=-1.0, scalar2=None, op0=Alu.mult)
                nc.scalar.activation(out=lexp[:], in_=ltan[:], func=Act.Exp,
                                     bias=nlmax[:, :1], scale=1.0, accum_out=lsum[:, :1])
                nc.scalar.activation(out=seg[:], in_=lsum[:], func=Act.Ln)
                nc.vector.tensor_tensor(out=llacc[:], in0=llacc[:], in1=seg[:], op=Alu.subtract)

                # 9) next gather index + prev bookkeeping
                nc.vector.tensor_tensor(out=idx_f[:], in0=iota101, in1=nxt_f[:], op=Alu.add)
                nc.vector.tensor_copy(out=idx_u[:], in_=idx_f[:])
                nc.vector.tensor_copy(out=prev_f[:], in_=nxt_f[:])
                nc.gpsimd.tensor_copy(out=idx_g[:], in_=idx_u[:])

            # cancel the spurious t=0 segment dist(depot, depot)=sqrt(1e-10)
            # exactly, by initializing cost to the identically-computed value
            # negated.
            nc.vector.memset(seg[:], 1e-10)
            nc.scalar.activation(out=seg[:], in_=seg[:], func=Act.Ln)
            nc.scalar.activation(out=seg[:], in_=seg[:], func=Act.Exp, bias=0.0, scale=0.5)
            nc.vector.tensor_scalar(out=costacc[:], in0=seg[:], scalar1=-1.0, scalar2=None, op0=Alu.mult)

            with tc.For_i(0, T - 2, 4):
                step_body()
                step_body()
                step_body()
                step_body()
            step_body()
            step_body()

            # epilogue: gather last-selected node's xy, add final tour
            # segment, then close to depot.
            nc.gpsimd.indirect_dma_start(
                out=g4[:], out_offset=None, in_=xyd_in[:],
                in_offset=bass.IndirectOffsetOnAxis(ap=idx_g[:, :1], axis=0))
            dist_to(g4[:, 0:2], costacc)
            nc.vector.tensor_copy(out=prevxy[:], in_=g4[:, 0:2])
            dist_to(depot, costacc)
            ostage = sp.tile([BC, 2], fp32)
            nc.vector.tensor_copy(out=ostage[:, 0:1], in_=costacc[:])
            nc.vector.tensor_copy(out=ostage[:, 1:2], in_=llacc[:])
            nc.sync.dma_start(out=out_dram[:], in_=ostage[:])

    nc.compile()
    return nc


def make_in_maps(inputs):
    """Host prep: slicing/layout only -- no GEMMs, no float64."""
    f4 = np.float32

    def as32(x):
        x = np.asarray(x)
        return x.astype(f4) if x.dtype != f4 else x

    ne = np.ascontiguousarray(as32(inputs["node_embeddings"])).reshape(B, N * E)
    ge = as32(inputs["graph_embedding"])
    wk1 = as32(inputs["Wk1"])
    wv = as32(inputs["Wv"])
    wk2t = np.ascontiguousarray(as32(inputs["Wk2"]).T)
    woutt = np.ascontiguousarray(as32(inputs["Wout"]).T)
    wqf = as32(inputs["Wq_fixed"])
    wqs = as32(inputs["Wq_step"])
    wqse = np.ascontiguousarray(wqs[:E])
    wql = np.ascontiguousarray(wqs[E:E + 1])
    depot = as32(inputs["depot_xy"])
    cxy = as32(inputs["customer_xy"])
    dem = as32(inputs["demand"])

    xyd = np.zeros((B, N, 4), f4)
    xyd[:, 0, 0:2] = depot
    xyd[:, 1:, 0:2] = cxy
    xyd[:, 1:, 2] = dem

    dyn = np.zeros((B, DYNW), f4)
    dyn[:, 0:100] = dem
    dyn[:, 100:102] = depot
    dyn[:, 102:202] = np.arange(1, N, dtype=f4)[None, :]
    dyn[:, 202] = np.tile(np.arange(BC, dtype=f4) * N, NCORES)
    dyn[:, 203] = NEGBIG          # mask0: depot masked at t=0
    dyn[:, 304] = 1.0

    in_maps = []
    for c in range(NCORES):
        s = slice(c * BC, (c + 1) * BC)
        in_maps.append({
            "ne": ne[s],
            "ge": ge[s],
            "wk1": wk1, "wv": wv, "wk2t": wk2t, "woutt": woutt,
            "wqf": wqf, "wqse": wqse, "wql": wql,
            "xyd": xyd[s].reshape(BC * N, 4),
            "dyn": dyn[s],
        })
    return in_maps


def _enable_jax_compilation_cache():
    # Persistent XLA compilation cache: run_bass_kernel_spmd re-jits a fresh
    # closure every call, so without this each warm call pays a full backend
    # re-compile (~0.3s) for an identical HLO module.
    try:
        import jax
        jax.config.update("jax_compilation_cache_dir", "/tmp/.bass_jax_comp_cache")
        jax.config.update("jax_persistent_cache_min_entry_size_bytes", -1)
        jax.config.update("jax_persistent_cache_min_compile_time_secs", 0)
    except Exception:
        pass


def kernel(**inputs):
    from concourse.bass_utils import run_bass_kernel_spmd

    _enable_jax_compilation_cache()
    cold = "nc" not in _COMPILED
    if cold:
        _COMPILED["nc"] = build_nc()
    nc = _COMPILED["nc"]

    in_maps = make_in_maps(inputs)
    if cold:
        # Warm XLA/PJRT/tunnel caches on the (uncounted) compile call so
        # subsequent calls are fast and stable.
        try:
            run_bass_kernel_spmd(nc, in_maps, list(range(NCORES)))
        except Exception:
            pass
    try:
        res = run_bass_kernel_spmd(nc, in_maps, list(range(NCORES)))
    except Exception:
        res = run_bass_kernel_spmd(nc, in_maps, list(range(NCORES)))
    outs = np.concatenate([np.asarray(res.results[c]["out"]) for c in range(NCORES)])
    return outs[:, 0].astype(np.float32).copy(), outs[:, 1].astype(np.float32).copy()


# revision 15
# speedup vs baseline: 1.4544x; 1.4544x over previous
"""VRP attention-decoder greedy-decode kernel for Trainium2 (Bass/Tile).

kernel(**inputs) takes the FULL unsharded inputs (B=1024) and returns
(cost[B], ll[B]) matching reference.reference().

Design ("batch-on-partition"): 8 NeuronCores x 128 instances; instance ==
SBUF partition.  All GEMM precompute (K1 = ne@Wk1, V = ne@Wv,
K2' = ne@(Wk2 Wout^T), NW = ne@Wq_step[:E] + ge@Wq_fixed) runs on-device
on the TensorEngine in a prologue -- per node: PE transpose of
ne[:, n, :] to [E, inst], then 4 matmuls with that block stationary --
so the host ships only the raw inputs (~7MB/core instead of ~27MB/core
of precomputed tables; host does no GEMMs at all).  NW rows are written
to a device DRAM scratch table and gathered per decode step by
prev-node index alongside a small [xy|demand] row gather.  The per-step
attention einsums are per-instance batched matvecs -> elementwise
products + pairwise-tree reductions on DVE/GPSIMD, split across both
engines by free-dim ranges.  argmax runs on masked pre-tanh logits
(tanh monotone + positive scaling), softmax uses per-head max shift and
reciprocal normalization, tanh and sqrt are computed via exp/ln so a
single ACT table set is used in-loop.
"""

import numpy as np

B = 1024
NCORES = 8
BC = B // NCORES          # 128 instances per core == SBUF partitions
N_CUST = 100
N = N_CUST + 1            # 101
E = 128
H = 8
DH = 16
T = 2 * N                 # 202
CLIP = 10.0
ISD = 1.0 / np.sqrt(DH)
ISE = 1.0 / np.sqrt(E)
NEGBIG = -1.0e9
NE_CH = 4                 # nodes per ne streaming chunk

# dyn layout: [0:100] dem | [100:102] depot | [102:202] iota_nodes |
# [202] 101*i | [203:304] mask0 | [304] 1.0
DYNW = 305

_COMPILED = {}


def build_nc():
    import concourse.bass as bass
    import concourse.bacc as bacc
    import concourse.mybir as mybir
    from concourse.tile import TileContext
    from concourse import masks

    fp32 = mybir.dt.float32
    Alu = mybir.AluOpType
    Act = mybir.ActivationFunctionType

    nc = bacc.Bacc()

    ne_in = nc.dram_tensor("ne", [BC, N * E], fp32, kind="ExternalInput")
    ge_in = nc.dram_tensor("ge", [BC, E], fp32, kind="ExternalInput")
    wk1_in = nc.dram_tensor("wk1", [E, E], fp32, kind="ExternalInput")
    wv_in = nc.dram_tensor("wv", [E, E], fp32, kind="ExternalInput")
    wk2t_in = nc.dram_tensor("wk2t", [E, E], fp32, kind="ExternalInput")
    woutt_in = nc.dram_tensor("woutt", [E, E], fp32, kind="ExternalInput")
    wqf_in = nc.dram_tensor("wqf", [E, E], fp32, kind="ExternalInput")
    wqse_in = nc.dram_tensor("wqse", [E, E], fp32, kind="ExternalInput")
    wql_in = nc.dram_tensor("wql", [1, E], fp32, kind="ExternalInput")
    xyd_in = nc.dram_tensor("xyd", [BC * N, 4], fp32, kind="ExternalInput")
    dyn_in = nc.dram_tensor("dyn", [BC, DYNW], fp32, kind="ExternalInput")

    out_dram = nc.dram_tensor("out", [BC, 2], fp32, kind="ExternalOutput")

    with TileContext(nc) as tc:
        with (
            tc.tile_pool(name="tables", bufs=1) as tp,
            tc.tile_pool(name="state", bufs=1) as sp,
            tc.tile_pool(name="scratch", bufs=1) as cp,
            tc.tile_pool(name="nestream", bufs=3) as npool,
            tc.tile_pool(name="statpool", bufs=3) as stpool,
            tc.tile_pool(name="ppre", bufs=1, space="PSUM") as ppre,
            tc.tile_pool(name="ptp", bufs=2, space="PSUM") as ptp,
            tc.tile_pool(name="pmm", bufs=2, space="PSUM") as pmm,
            tc.tile_pool(name="dram", bufs=1, space="DRAM") as dpool,
        ):
            # ---- resident tables (155KB/partition), filled by prologue ----
            k1l = tp.tile([BC, H * N * DH], fp32)   # (h, n, d)
            vl = tp.tile([BC, H * DH * N], fp32)    # (h, d, n)
            k2l = tp.tile([BC, N * E], fp32)        # (n, e)
            nw_dram = dpool.tile([BC * N, E], fp32)

            # ---- small loads ----
            dyn = sp.tile([BC, DYNW], fp32)
            nc.sync.dma_start(out=dyn[:], in_=dyn_in[:])
            dem = dyn[:, 0:100]
            depot = dyn[:, 100:102]
            iota_nodes = dyn[:, 102:202]
            iota101 = dyn[:, 202:203]
            ones_col = dyn[:, 304:305]

            wk1 = tp.tile([E, E], fp32)
            nc.sync.dma_start(out=wk1[:], in_=wk1_in[:])
            wv = tp.tile([E, E], fp32)
            nc.sync.dma_start(out=wv[:], in_=wv_in[:])
            wk2t = tp.tile([E, E], fp32)
            nc.sync.dma_start(out=wk2t[:], in_=wk2t_in[:])
            woutt = tp.tile([E, E], fp32)
            nc.sync.dma_start(out=woutt[:], in_=woutt_in[:])
            wqf = tp.tile([E, E], fp32)
            nc.sync.dma_start(out=wqf[:], in_=wqf_in[:])
            wqse = tp.tile([E, E], fp32)
            nc.sync.dma_start(out=wqse[:], in_=wqse_in[:])
            wql = tp.tile([1, E], fp32)
            nc.sync.dma_start(out=wql[:], in_=wql_in[:])
            geb = tp.tile([BC, E], fp32)
            nc.sync.dma_start(out=geb[:], in_=ge_in[:])

            ident = tp.tile([E, E], fp32)
            masks.make_identity(nc, ident[:])
            ones1 = tp.tile([1, BC], fp32)
            nc.vector.memset(ones1[:], 1.0)

            # ---- prologue GEMMs ----
            # C = Wk2 @ Wout^T  (out[i,f] = sum_k Wk2[i,k] Wout[f,k])
            cps = ppre.tile([E, E], fp32, tag="cps")
            nc.tensor.matmul(out=cps[:], lhsT=wk2t[:], rhs=woutt[:], start=True, stop=True)
            crhs = tp.tile([E, E], fp32)
            nc.vector.tensor_copy(out=crhs[:], in_=cps[:])

            # wrep = ones(BC) x Wq_step[E]  (broadcast last row across partitions)
            wrp = ppre.tile([BC, E], fp32, tag="wrp")
            nc.tensor.matmul(out=wrp[:], lhsT=ones1[:], rhs=wql[:], start=True, stop=True)
            wrep = tp.tile([BC, E], fp32)
            nc.vector.tensor_copy(out=wrep[:], in_=wrp[:])

            # Qf = ge @ Wq_fixed
            gept = ppre.tile([E, BC], fp32, tag="gept")
            nc.tensor.transpose(out=gept[:], in_=geb[:], identity=ident[:])
            gets = tp.tile([E, BC], fp32)
            nc.scalar.activation(out=gets[:], in_=gept[:], func=Act.Copy)
            qfp = ppre.tile([BC, E], fp32, tag="qfp")
            nc.tensor.matmul(out=qfp[:], lhsT=gets[:], rhs=wqf[:], start=True, stop=True)
            qf = tp.tile([BC, E], fp32)
            nc.vector.tensor_copy(out=qf[:], in_=qfp[:])

            # per-node: transpose ne_n -> stationary; 4 matmuls; scatter to
            # table layouts; NW row (+Qf) out to DRAM gather table.
            k1v4 = k1l[:].rearrange("p (h n d) -> p h n d", h=H, n=N)
            vlv4 = vl[:].rearrange("p (h d n) -> p h d n", h=H, d=DH)
            nwv = nw_dram[:].rearrange("(b n) e -> b n e", n=N)
            for c0 in range(0, N, NE_CH):
                w = min(NE_CH, N - c0)
                neb = npool.tile([BC, NE_CH * E], fp32, tag="neb")
                nc.sync.dma_start(out=neb[:, 0:w * E], in_=ne_in[:, c0 * E:(c0 + w) * E])
                for k in range(w):
                    n = c0 + k
                    tps = ptp.tile([E, BC], fp32, tag="tps")
                    nc.tensor.transpose(out=tps[:], in_=neb[:, k * E:(k + 1) * E], identity=ident[:])
                    stat = stpool.tile([E, BC], fp32, tag="stat")
                    nc.scalar.activation(out=stat[:], in_=tps[:], func=Act.Copy)
                    mo = pmm.tile([BC, 4 * E], fp32, tag="mo")
                    nc.tensor.matmul(out=mo[:, 0:E], lhsT=stat[:], rhs=wk1[:], start=True, stop=True)
                    nc.tensor.matmul(out=mo[:, E:2 * E], lhsT=stat[:], rhs=wv[:], start=True, stop=True)
                    nc.tensor.matmul(out=mo[:, 2 * E:3 * E], lhsT=stat[:], rhs=crhs[:], start=True, stop=True)
                    nc.tensor.matmul(out=mo[:, 3 * E:4 * E], lhsT=stat[:], rhs=wqse[:], start=True, stop=True)
                    nc.vector.tensor_copy(
                        out=k1v4[:, :, n, :],
                        in_=mo[:, 0:E].rearrange("p (h d) -> p h d", h=H))
                    nc.scalar.activation(
                        out=vlv4[:, :, :, n],
                        in_=mo[:, E:2 * E].rearrange("p (h d) -> p h d", h=H),
                        func=Act.Copy)
                    nc.vector.tensor_copy(out=k2l[:, n * E:(n + 1) * E], in_=mo[:, 2 * E:3 * E])
                    nwst = stpool.tile([BC, E], fp32, tag="nwst")
                    nc.vector.tensor_tensor(out=nwst[:], in0=mo[:, 3 * E:4 * E], in1=qf[:], op=Alu.add)
                    nc.scalar.dma_start(out=nwv[:, n, :], in_=nwst[:])

            # ---- state ----
            maskneg = sp.tile([BC, N], fp32)
            nc.vector.tensor_copy(out=maskneg[:], in_=dyn[:, 203:304])
            visited = sp.tile([BC, N_CUST], fp32)
            nc.vector.memset(visited[:], 0.0)
            Dcap = sp.tile([BC, 1], fp32)
            nc.vector.tensor_copy(out=Dcap[:], in_=ones_col)
            llacc = sp.tile([BC, 1], fp32)
            nc.vector.memset(llacc[:], 0.0)
            costacc = sp.tile([BC, 1], fp32)
            prevxy = sp.tile([BC, 2], fp32)
            nc.vector.tensor_copy(out=prevxy[:], in_=depot)
            idx_f = sp.tile([BC, 1], fp32)
            nc.vector.tensor_copy(out=idx_f[:], in_=iota101)
            idx_u = sp.tile([BC, 1], mybir.dt.uint32)
            nc.vector.tensor_copy(out=idx_u[:], in_=idx_f[:])
            prev_f = sp.tile([BC, 1], fp32)
            nc.vector.memset(prev_f[:], 0.0)
            idx_g = sp.tile([BC, 1], mybir.dt.uint32)
            nc.gpsimd.tensor_copy(out=idx_g[:], in_=idx_u[:])

            # ---- shared per-step scratch (~38KB/partition) ----
            g128 = cp.tile([BC, E], fp32, tag="g128")
            g4 = cp.tile([BC, 4], fp32, tag="g4")
            q1 = cp.tile([BC, E], fp32, tag="q1")
            dterm = cp.tile([BC, E], fp32, tag="dterm")
            prod = cp.tile([BC, 3328], fp32, tag="prod")
            ta = cp.tile([BC, 1664], fp32, tag="ta")
            tb = cp.tile([BC, 832], fp32, tag="tb")
            tc_ = cp.tile([BC, 416], fp32, tag="tc_")
            td = cp.tile([BC, 232], fp32, tag="td")
            te = cp.tile([BC, 128], fp32, tag="te")
            tf = cp.tile([BC, 64], fp32, tag="tf")
            scor = cp.tile([BC, H * N], fp32, tag="scor")
            uexp = cp.tile([BC, H * N], fp32, tag="uexp")
            ssum = cp.tile([BC, H], fp32, tag="ssum")
            srec = cp.tile([BC, H], fp32, tag="srec")
            nsc = cp.tile([BC, H], fp32, tag="nsc")
            hmax = cp.tile([BC, H], fp32, tag="hmax")
            glm = cp.tile([BC, E], fp32, tag="glm")
            raw = cp.tile([BC, N], fp32, tag="raw")
            mx8 = cp.tile([BC, 8], fp32, tag="mx8")
            nxt8 = cp.tile([BC, 8], mybir.dt.uint32, tag="nxt8")
            nxt_f = cp.tile([BC, 1], fp32, tag="nxt_f")
            ltan = cp.tile([BC, N], fp32, tag="ltan")
            lexp = cp.tile([BC, N], fp32, tag="lexp")
            lsum = cp.tile([BC, 1], fp32, tag="lsum")
            lmax = cp.tile([BC, 1], fp32, tag="lmax")
            nlmax = cp.tile([BC, 1], fp32, tag="nlmax")
            tiny = cp.tile([BC, 2], fp32, tag="tiny")
            seg = cp.tile([BC, 1], fp32, tag="seg")
            oh = cp.tile([BC, N_CUST], fp32, tag="oh")
            gtd = cp.tile([BC, N_CUST], fp32, tag="gtd")
            sdep = cp.tile([BC, 1], fp32, tag="sdep")
            sdep_i = cp.tile([BC, 1], mybir.dt.int32, tag="sdep_i")
            av = cp.tile([BC, 1], fp32, tag="av")
            dnew = cp.tile([BC, 1], fp32, tag="dnew")

            def dist_to(xyap, acc):
                nc.vector.tensor_tensor(out=tiny[:], in0=xyap, in1=prevxy[:], op=Alu.subtract)
                nc.vector.tensor_tensor(out=tiny[:], in0=tiny[:], in1=tiny[:], op=Alu.mult)
                nc.vector.tensor_reduce(out=seg[:], in_=tiny[:, None, :], axis=mybir.AxisListType.X, op=Alu.add)
                nc.vector.tensor_scalar(out=seg[:], in0=seg[:], scalar1=1e-10, scalar2=None, op0=Alu.add)
                nc.scalar.activation(out=seg[:], in_=seg[:], func=Act.Ln)
                nc.scalar.activation(out=seg[:], in_=seg[:], func=Act.Exp, bias=0.0, scale=0.5)
                nc.vector.tensor_tensor(out=acc[:], in0=acc[:], in1=seg[:], op=Alu.add)

            def step_body():
                # 1) gather [xy | dem] and Q1-part rows by prev (last-selected) index
                nc.gpsimd.indirect_dma_start(
                    out=g4[:], out_offset=None, in_=xyd_in[:],
                    in_offset=bass.IndirectOffsetOnAxis(ap=idx_g[:, :1], axis=0))
                nc.gpsimd.indirect_dma_start(
                    out=g128[:], out_offset=None, in_=nw_dram[:],
                    in_offset=bass.IndirectOffsetOnAxis(ap=idx_g[:, :1], axis=0))

                # 1b) deferred env update for the node selected last step.
                #     At t=0 prev=depot and this exactly reproduces the
                #     reference initial state (given visited=0, D=1).
                nc.vector.tensor_scalar(out=sdep[:], in0=prev_f[:], scalar1=0.0, scalar2=None, op0=Alu.is_equal)
                nc.vector.tensor_copy(out=sdep_i[:], in_=sdep[:])
                nc.vector.tensor_tensor(out=dnew[:], in0=Dcap[:], in1=g4[:, 2:3], op=Alu.subtract)
                nc.vector.select(out=Dcap[:], mask=sdep_i[:], on_true=ones_col, on_false=dnew[:])
                nc.vector.tensor_scalar(out=oh[:], in0=iota_nodes, scalar1=prev_f[:, :1], scalar2=None, op0=Alu.is_equal)
                nc.vector.tensor_tensor(out=visited[:], in0=visited[:], in1=oh[:], op=Alu.max)
                nc.vector.tensor_scalar(out=gtd[:], in0=dem, scalar1=Dcap[:, :1], scalar2=None, op0=Alu.is_gt)
                nc.vector.tensor_tensor(out=gtd[:], in0=gtd[:], in1=visited[:], op=Alu.max)
                nc.vector.tensor_scalar(out=maskneg[:, 1:N], in0=gtd[:], scalar1=float(NEGBIG), scalar2=None, op0=Alu.mult)
                nc.vector.tensor_reduce(out=av[:], in_=visited[:], axis=mybir.AxisListType.X, op=Alu.min)
                nc.vector.tensor_scalar(out=av[:], in0=av[:], scalar1=-1.0, scalar2=1.0, op0=Alu.mult, op1=Alu.add)
                nc.vector.tensor_tensor(out=av[:], in0=av[:], in1=sdep[:], op=Alu.mult)
                nc.vector.tensor_scalar(out=maskneg[:, 0:1], in0=av[:], scalar1=float(NEGBIG), scalar2=None, op0=Alu.mult)

                # 1c) deferred cost segment to the last-selected node
                dist_to(g4[:, 0:2], costacc)
                nc.vector.tensor_copy(out=prevxy[:], in_=g4[:, 0:2])

                # 2) Q1 = gathered + D * w_last
                nc.vector.tensor_scalar(out=dterm[:], in0=wrep[:], scalar1=Dcap[:, :1],
                                        scalar2=None, op0=Alu.mult)
                nc.vector.tensor_tensor(out=q1[:], in0=g128[:], in1=dterm[:], op=Alu.add)

                # 3) scores, head-pair chunks: K1L[h,n,d]*Q1[h,d] -> sum_d
                q1v = q1[:].rearrange("p (h d) -> p h d", h=H)
                k1v = k1l[:].rearrange("p (h n d) -> p h n d", h=H, n=N)
                p1v = prod[:, 0:2 * N * DH].rearrange("p (h n d) -> p h n d", h=2, n=N)
                for hp in range(4):
                    h0 = 2 * hp
                    qs = q1v[:, h0:h0 + 2, None, :].to_broadcast([BC, 2, 68, DH])
                    nc.vector.tensor_tensor(out=p1v[:, :, 0:68, :],
                                            in0=k1v[:, h0:h0 + 2, 0:68, :], in1=qs, op=Alu.mult)
                    qs2 = q1v[:, h0:h0 + 2, None, :].to_broadcast([BC, 2, 33, DH])
                    nc.gpsimd.tensor_tensor(out=p1v[:, :, 68:N, :],
                                            in0=k1v[:, h0:h0 + 2, 68:N, :], in1=qs2, op=Alu.mult)
                    a = prod[:, 0:2 * N * DH].rearrange("p (x d) -> p x d", d=DH)   # x=202
                    r1 = ta[:, 0:202 * 8].rearrange("p (x d) -> p x d", d=8)
                    nc.vector.tensor_tensor(out=r1[:, 0:140, :], in0=a[:, 0:140, 0:8], in1=a[:, 0:140, 8:16], op=Alu.add)
                    nc.gpsimd.tensor_tensor(out=r1[:, 140:202, :], in0=a[:, 140:202, 0:8], in1=a[:, 140:202, 8:16], op=Alu.add)
                    r2 = tb[:, 0:202 * 4].rearrange("p (x d) -> p x d", d=4)
                    nc.vector.tensor_tensor(out=r2[:, 0:140, :], in0=r1[:, 0:140, 0:4], in1=r1[:, 0:140, 4:8], op=Alu.add)
                    nc.gpsimd.tensor_tensor(out=r2[:, 140:202, :], in0=r1[:, 140:202, 0:4], in1=r1[:, 140:202, 4:8], op=Alu.add)
                    r3 = tc_[:, 0:202 * 2].rearrange("p (x d) -> p x d", d=2)
                    nc.vector.tensor_tensor(out=r3[:, :, :], in0=r2[:, :, 0:2], in1=r2[:, :, 2:4], op=Alu.add)
                    nc.vector.tensor_tensor(
                        out=scor[:, h0 * N:(h0 + 2) * N].rearrange("p (x o) -> p x o", o=1),
                        in0=r3[:, :, 0:1], in1=r3[:, :, 1:2], op=Alu.add)

                # 4) mask + per-head exp (accumulating denominator) + reciprocal
                nc.vector.tensor_tensor(
                    out=scor[:].rearrange("p (h n) -> p h n", h=H),
                    in0=scor[:].rearrange("p (h n) -> p h n", h=H),
                    in1=maskneg[:, None, :].to_broadcast([BC, H, N]), op=Alu.add)
                nc.vector.tensor_reduce(
                    out=hmax[:], in_=scor[:].rearrange("p (h n) -> p h n", h=H),
                    axis=mybir.AxisListType.X, op=Alu.max)
                nc.vector.tensor_scalar(out=hmax[:], in0=hmax[:], scalar1=float(-ISD), scalar2=None, op0=Alu.mult)
                for h in range(H):
                    nc.scalar.activation(out=uexp[:, h * N:(h + 1) * N],
                                         in_=scor[:, h * N:(h + 1) * N],
                                         func=Act.Exp, bias=hmax[:, h:h + 1], scale=float(ISD),
                                         accum_out=ssum[:, h:h + 1])
                nc.vector.reciprocal(out=srec[:], in_=ssum[:])
                nc.vector.tensor_tensor(out=nsc[:], in0=ssum[:], in1=srec[:], op=Alu.mult)
                nc.vector.tensor_scalar(out=nsc[:], in0=nsc[:], scalar1=-1.0, scalar2=2.0, op0=Alu.mult, op1=Alu.add)
                nc.vector.tensor_tensor(out=srec[:], in0=srec[:], in1=nsc[:], op=Alu.mult)

                # 5) glimpse, head-pair chunks: VL[h,d,n]*U[h,n] -> sum_n
                vlv = vl[:].rearrange("p (h d n) -> p h d n", h=H, d=DH)
                uv = uexp[:].rearrange("p (h n) -> p h n", h=H)
                p2v = prod[:, 0:2 * DH * N].rearrange("p (h d n) -> p h d n", h=2, d=DH)
                for hp in range(4):
                    h0 = 2 * hp
                    us = uv[:, h0:h0 + 2, None, 0:68].to_broadcast([BC, 2, DH, 68])
                    nc.vector.tensor_tensor(out=p2v[:, :, :, 0:68],
                                            in0=vlv[:, h0:h0 + 2, :, 0:68], in1=us, op=Alu.mult)
                    us2 = uv[:, h0:h0 + 2, None, 68:N].to_broadcast([BC, 2, DH, 33])
                    nc.gpsimd.tensor_tensor(out=p2v[:, :, :, 68:N],
                                            in0=vlv[:, h0:h0 + 2, :, 68:N], in1=us2, op=Alu.mult)
                    # n-tree: 101 -> 51 -> 26 -> 13 -> 7 -> 4 -> 2 -> 1  (x = 32 rows)
                    a = prod[:, 0:2 * DH * N].rearrange("p (x n) -> p x n", n=N)
                    r1 = ta[:, 0:32 * 51].rearrange("p (x n) -> p x n", n=51)
                    nc.vector.tensor_tensor(out=r1[:, 0:20, 0:50], in0=a[:, 0:20, 0:50], in1=a[:, 0:20, 50:100], op=Alu.add)
                    nc.gpsimd.tensor_tensor(out=r1[:, 20:32, 0:50], in0=a[:, 20:32, 0:50], in1=a[:, 20:32, 50:100], op=Alu.add)
                    nc.vector.tensor_copy(out=r1[:, :, 50:51], in_=a[:, :, 100:101])
                    r2 = tb[:, 0:32 * 26].rearrange("p (x n) -> p x n", n=26)
                    nc.vector.tensor_tensor(out=r2[:, :, 0:25], in0=r1[:, :, 0:25], in1=r1[:, :, 25:50], op=Alu.add)
                    nc.vector.tensor_copy(out=r2[:, :, 25:26], in_=r1[:, :, 50:51])
                    r3 = tc_[:, 0:32 * 13].rearrange("p (x n) -> p x n", n=13)
                    nc.vector.tensor_tensor(out=r3[:, :, :], in0=r2[:, :, 0:13], in1=r2[:, :, 13:26], op=Alu.add)
                    r4 = td[:, 0:32 * 7].rearrange("p (x n) -> p x n", n=7)
                    nc.vector.tensor_tensor(out=r4[:, :, 0:6], in0=r3[:, :, 0:6], in1=r3[:, :, 6:12], op=Alu.add)
                    nc.vector.tensor_copy(out=r4[:, :, 6:7], in_=r3[:, :, 12:13])
                    r5 = te[:, 0:32 * 4].rearrange("p (x n) -> p x n", n=4)
                    nc.vector.tensor_tensor(out=r5[:, :, 0:3], in0=r4[:, :, 0:3], in1=r4[:, :, 3:6], op=Alu.add)
                    nc.vector.tensor_copy(out=r5[:, :, 3:4], in_=r4[:, :, 6:7])
                    r6 = tf[:, 0:32 * 2].rearrange("p (x n) -> p x n", n=2)
                    nc.vector.tensor_tensor(out=r6[:, :, :], in0=r5[:, :, 0:2], in1=r5[:, :, 2:4], op=Alu.add)
                    nc.vector.tensor_tensor(
                        out=glm[:, h0 * DH:(h0 + 2) * DH].rearrange("p (x o) -> p x o", o=1),
                        in0=r6[:, :, 0:1], in1=r6[:, :, 1:2], op=Alu.add)
                # normalize glimpse per head
                nc.vector.tensor_tensor(
                    out=glm[:].rearrange("p (h d) -> p h d", h=H),
                    in0=glm[:].rearrange("p (h d) -> p h d", h=H),
                    in1=srec[:, :, None].to_broadcast([BC, H, DH]), op=Alu.mult)

                # 6) logits, n'-chunks of 26: K2L[n',e]*G[e] -> sum_e
                k2v = k2l[:].rearrange("p (n e) -> p n e", n=N)
                for c in range(4):
                    n0 = 26 * c
                    n1 = min(N, n0 + 26)
                    w = n1 - n0
                    gb = glm[:, None, :].to_broadcast([BC, w, E])
                    p3v = prod[:, 0:w * E].rearrange("p (n e) -> p n e", e=E)
                    nc.vector.tensor_tensor(out=p3v[:, :, :], in0=k2v[:, n0:n1, :], in1=gb, op=Alu.mult)
                    r1 = ta[:, 0:w * 64].rearrange("p (n e) -> p n e", e=64)
                    hw = (w * 2) // 3
                    nc.vector.tensor_tensor(out=r1[:, 0:hw, :], in0=p3v[:, 0:hw, 0:64], in1=p3v[:, 0:hw, 64:128], op=Alu.add)
                    nc.gpsimd.tensor_tensor(out=r1[:, hw:w, :], in0=p3v[:, hw:w, 0:64], in1=p3v[:, hw:w, 64:128], op=Alu.add)
                    r2 = tb[:, 0:w * 32].rearrange("p (n e) -> p n e", e=32)
                    nc.vector.tensor_tensor(out=r2[:, :, :], in0=r1[:, :, 0:32], in1=r1[:, :, 32:64], op=Alu.add)
                    r3 = tc_[:, 0:w * 16].rearrange("p (n e) -> p n e", e=16)
                    nc.vector.tensor_tensor(out=r3[:, :, :], in0=r2[:, :, 0:16], in1=r2[:, :, 16:32], op=Alu.add)
                    r4 = td[:, 0:w * 8].rearrange("p (n e) -> p n e", e=8)
                    nc.vector.tensor_tensor(out=r4[:, :, :], in0=r3[:, :, 0:8], in1=r3[:, :, 8:16], op=Alu.add)
                    r5 = te[:, 0:w * 4].rearrange("p (n e) -> p n e", e=4)
                    nc.vector.tensor_tensor(out=r5[:, :, :], in0=r4[:, :, 0:4], in1=r4[:, :, 4:8], op=Alu.add)
                    r6 = tf[:, 0:w * 2].rearrange("p (n e) -> p n e", e=2)
                    nc.vector.tensor_tensor(out=r6[:, :, :], in0=r5[:, :, 0:2], in1=r5[:, :, 2:4], op=Alu.add)
                    nc.vector.tensor_tensor(
                        out=raw[:, n0:n1].rearrange("p (n o) -> p n o", o=1),
                        in0=r6[:, :, 0:1], in1=r6[:, :, 1:2], op=Alu.add)

                # 7) mask + argmax on pre-tanh logits
                nc.vector.tensor_tensor(out=raw[:], in0=raw[:], in1=maskneg[:], op=Alu.add)
                nc.vector.max(out=mx8[:], in_=raw[:])
                nc.vector.max_index(out=nxt8[:], in_max=mx8[:], in_values=raw[:])
                nc.vector.tensor_copy(out=nxt_f[:], in_=nxt8[:, 0:1])

                # 8) next gather index + prev bookkeeping FIRST: gets idx_g
                #    onto the gpsimd queue early so the next step's gathers
                #    overlap the ll tail below (blocks are data-disjoint).
                nc.vector.tensor_tensor(out=idx_f[:], in0=iota101, in1=nxt_f[:], op=Alu.add)
                nc.vector.tensor_copy(out=idx_u[:], in_=idx_f[:])
                nc.vector.tensor_copy(out=prev_f[:], in_=nxt_f[:])
                nc.gpsimd.tensor_copy(out=idx_g[:], in_=idx_u[:])

                # 9) ll: L = CLIP*tanh(ISE*rawu) + maskNEG; tanh via exp.
                nc.vector.tensor_tensor(out=ltan[:], in0=raw[:], in1=maskneg[:], op=Alu.subtract)
                nc.scalar.activation(out=lexp[:], in_=ltan[:], func=Act.Exp,
                                     bias=0.0, scale=float(2.0 * ISE))
                nc.vector.tensor_scalar(out=lexp[:], in0=lexp[:], scalar1=1.0, scalar2=None, op0=Alu.add)
                nc.vector.reciprocal(out=lexp[:], in_=lexp[:])
                nc.vector.tensor_scalar(out=ltan[:], in0=lexp[:], scalar1=-2.0 * CLIP, scalar2=CLIP, op0=Alu.mult, op1=Alu.add)
                nc.vector.tensor_tensor(out=ltan[:], in0=ltan[:], in1=maskneg[:], op=Alu.add)
                nc.vector.tensor_reduce(out=lmax[:], in_=ltan[:], axis=mybir.AxisListType.X, op=Alu.max)
                nc.vector.tensor_scalar(out=nlmax[:], in0=lmax[:], scalar1=-1.0, scalar2=None, op0=Alu.mult)
                nc.scalar.activation(out=lexp[:], in_=ltan[:], func=Act.Exp,
                                     bias=nlmax[:, :1], scale=1.0, accum_out=lsum[:, :1])
                nc.scalar.activation(out=seg[:], in_=lsum[:], func=Act.Ln)
                nc.vector.tensor_tensor(out=llacc[:], in0=llacc[:], in1=seg[:], op=Alu.subtract)

            # cancel the spurious t=0 segment dist(depot, depot)=sqrt(1e-10)
            # exactly, by initializing cost to the identically-computed value
            # negated.
            nc.vector.memset(seg[:], 1e-10)
            nc.scalar.activation(out=seg[:], in_=seg[:], func=Act.Ln)
            nc.scalar.activation(out=seg[:], in_=seg[:], func=Act.Exp, bias=0.0, scale=0.5)
            nc.vector.tensor_scalar(out=costacc[:], in0=seg[:], scalar1=-1.0, scalar2=None, op0=Alu.mult)

            with tc.For_i(0, T - 2, 4):
                step_body()
                step_body()
                step_body()
                step_body()
            step_body()
            step_body()

            # epilogue: gather last-selected node's xy, add final tour
            # segment, then close to depot.
            nc.gpsimd.indirect_dma_start(
                out=g4[:], out_offset=None, in_=xyd_in[:],
                in_offset=bass.IndirectOffsetOnAxis(ap=idx_g[:, :1], axis=0))
            dist_to(g4[:, 0:2], costacc)
            nc.vector.tensor_copy(out=prevxy[:], in_=g4[:, 0:2])
            dist_to(depot, costacc)
            ostage = sp.tile([BC, 2], fp32)
            nc.vector.tensor_copy(out=ostage[:, 0:1], in_=costacc[:])
            nc.vector.tensor_copy(out=ostage[:, 1:2], in_=llacc[:])
            nc.sync.dma_start(out=out_dram[:], in_=ostage[:])

    nc.compile()
    return nc


def make_in_maps(inputs):
    """Host prep: slicing/layout only -- no GEMMs, no float64."""
    f4 = np.float32

    def as32(x):
        x = np.asarray(x)
        return x.astype(f4) if x.dtype != f4 else x

    ne = np.ascontiguousarray(as32(inputs["node_embeddings"])).reshape(B, N * E)
    ge = as32(inputs["graph_embedding"])
    wk1 = as32(inputs["Wk1"])
    wv = as32(inputs["Wv"])
    wk2t = np.ascontiguousarray(as32(inputs["Wk2"]).T)
    woutt = np.ascontiguousarray(as32(inputs["Wout"]).T)
    wqf = as32(inputs["Wq_fixed"])
    wqs = as32(inputs["Wq_step"])
    wqse = np.ascontiguousarray(wqs[:E])
    wql = np.ascontiguousarray(wqs[E:E + 1])
    depot = as32(inputs["depot_xy"])
    cxy = as32(inputs["customer_xy"])
    dem = as32(inputs["demand"])

    xyd = np.zeros((B, N, 4), f4)
    xyd[:, 0, 0:2] = depot
    xyd[:, 1:, 0:2] = cxy
    xyd[:, 1:, 2] = dem

    dyn = np.zeros((B, DYNW), f4)
    dyn[:, 0:100] = dem
    dyn[:, 100:102] = depot
    dyn[:, 102:202] = np.arange(1, N, dtype=f4)[None, :]
    dyn[:, 202] = np.tile(np.arange(BC, dtype=f4) * N, NCORES)
    dyn[:, 203] = NEGBIG          # mask0: depot masked at t=0
    dyn[:, 304] = 1.0

    in_maps = []
    for c in range(NCORES):
        s = slice(c * BC, (c + 1) * BC)
        in_maps.append({
            "ne": ne[s],
            "ge": ge[s],
            "wk1": wk1, "wv": wv, "wk2t": wk2t, "woutt": woutt,
            "wqf": wqf, "wqse": wqse, "wql": wql,
            "xyd": xyd[s].reshape(BC * N, 4),
            "dyn": dyn[s],
        })
    return in_maps


def _enable_jax_compilation_cache():
    # Persistent XLA compilation cache: run_bass_kernel_spmd re-jits a fresh
    # closure every call, so without this each warm call pays a full backend
    # re-compile (~0.3s) for an identical HLO module.
    try:
        import jax
        jax.config.update("jax_compilation_cache_dir", "/tmp/.bass_jax_comp_cache")
        jax.config.update("jax_persistent_cache_min_entry_size_bytes", -1)
        jax.config.update("jax_persistent_cache_min_compile_time_secs", 0)
    except Exception:
        pass


def kernel(**inputs):
    from concourse.bass_utils import run_bass_kernel_spmd

    _enable_jax_compilation_cache()
    cold = "nc" not in _COMPILED
    if cold:
        _COMPILED["nc"] = build_nc()
    nc = _COMPILED["nc"]

    in_maps = make_in_maps(inputs)
    if cold:
        # Warm XLA/PJRT/tunnel caches on the (uncounted) compile call so
        # subsequent calls are fast and stable.
        try:
            run_bass_kernel_spmd(nc, in_maps, list(range(NCORES)))
        except Exception:
            pass
    try:
        res = run_bass_kernel_spmd(nc, in_maps, list(range(NCORES)))
    except Exception:
        res = run_bass_kernel_spmd(nc, in_maps, list(range(NCORES)))
    outs = np.concatenate([np.asarray(res.results[c]["out"]) for c in range(NCORES)])
    return outs[:, 0].astype(np.float32).copy(), outs[:, 1].astype(np.float32).copy()
